# revision 33
# baseline (speedup 1.0000x reference)
"""Trainium2 Bass kernel for nn_MoE_42984032698463 (4-gate MoE over 4 Mamba experts).

Sharding: 16 (expert, batch) pairs -> 8 cores, each core owns 1 expert x 2 batches.
Device per pair: in_proj (PE f32r) -> causal depthwise conv (PE diag-shift matmuls)
-> silu -> x_proj -> dt_proj/softplus -> selective scan via native tensor_tensor_scan
(bf16) -> gated output -> fused out_proj@proj (PE). Host: LayerNorm fold, routing
softmax/top-k, weighted combine, aux loss.
"""
import numpy as np
import ml_dtypes

import concourse.bacc as bacc
import concourse.bass as bass
import concourse.tile as tile
from concourse import mybir
from concourse.bass_utils import run_bass_kernel_spmd

F32 = mybir.dt.float32
F32R = mybir.dt.float32r
BF16 = mybir.dt.bfloat16
AF = mybir.ActivationFunctionType
OP = mybir.AluOpType

# problem dims (hardcoded per contract)
E, B, C, Hh, Ww = 4, 4, 128, 32, 32
L = Hh * Ww            # 1024
DIN = 2 * C            # 256
S, RK, KC = 16, 8, 4   # d_state, dt_rank, d_conv
TOPK = 2
NCORES = 8
P = 128                # partitions
NT = DIN // P          # 2 d-tiles
NCH = L // 512         # 2 N-chunks per matmul row


def _r(ap):
    return ap  # f32r needs producer-side rounding; plain fp32 for now


def _patch_act_tables():
    """Make Exp and Ln resolve to the combined natural_log_exp set (one table
    load instead of thrashing exp_and_others <-> natural_log per softplus).
    Keeps dict order/length so act_func_set_id indices stay valid."""
    if _CACHE.get("tables_patched"):
        return
    import concourse.bacc as _bacc
    real = _bacc.get_activation_tables

    def patched(arch):
        tabs = dict(real(arch))
        if "natural_log_exp_and_others" in tabs:
            for name in ("exp_and_others", "natural_log", "exp_and_friends"):
                if name in tabs:
                    tabs[name] = set()
        return tabs

    _bacc.get_activation_tables = patched
    _CACHE["tables_patched"] = True


def build_program(fused_exp=False):
    _patch_act_tables()
    nc = bacc.Bacc("TRN2", target_bir_lowering=False, debug=False)

    # ---- DRAM I/O (per core) ----
    d_xh = nc.dram_tensor("xh", [P, 2, L], F32R, kind="ExternalInput")
    d_wxt = nc.dram_tensor("wxt", [P, DIN], F32R, kind="ExternalInput")
    d_wzt = nc.dram_tensor("wzt", [P, DIN], F32R, kind="ExternalInput")
    d_bias4 = nc.dram_tensor("bias4", [P, 4], F32, kind="ExternalInput")
    d_convd = nc.dram_tensor("convd", [P, NT * KC, P], F32R, kind="ExternalInput")
    d_convb = nc.dram_tensor("convb", [P, NT], F32, kind="ExternalInput")
    d_xprojt = nc.dram_tensor("xprojt", [P, NT, RK + 2 * S], F32R, kind="ExternalInput")
    d_dtprojt = nc.dram_tensor("dtprojt", [RK, DIN], F32R, kind="ExternalInput")
    d_dtb = nc.dram_tensor("dtb", [P, NT], F32, kind="ExternalInput")
    d_acol = nc.dram_tensor("acol", [P, NT * S], F32, kind="ExternalInput")
    d_dcol = nc.dram_tensor("dcol", [P, NT], F32, kind="ExternalInput")
    d_wopt = nc.dram_tensor("wopt", [P, NT, P], F32R, kind="ExternalInput")
    d_ident = nc.dram_tensor("ident", [P, P], BF16, kind="ExternalInput")
    d_out = [nc.dram_tensor(f"o{p}", [P, L], F32, kind="ExternalOutput")
             for p in range(2)]

    with tile.TileContext(nc) as tc:
        with (
            tc.tile_pool(name="consts", bufs=1) as consts,
            tc.tile_pool(name="acts", bufs=1) as acts,
            tc.tile_pool(name="psum", bufs=2, space="PSUM") as psum,
            tc.tile_pool(name="ypsum", bufs=1, space="PSUM") as ypsum,
            tc.tile_pool(name="brep", bufs=3) as brep_pool,
            tc.tile_pool(name="crep", bufs=3) as crep_pool,
            tc.tile_pool(name="sblk", bufs=3) as sblk,
            tc.tile_pool(name="hblk", bufs=3) as hblk,
            tc.tile_pool(name="outp", bufs=2) as outp,
            tc.tile_pool(name="dramp", bufs=2, space="DRAM") as dramp,
        ):
            # ---- load constants ----
            wxt = consts.tile([P, DIN], F32R)
            wzt = consts.tile([P, DIN], F32R)
            bias4 = consts.tile([P, 4], F32)
            convd = consts.tile([P, NT * KC, P], F32R)
            convb = consts.tile([P, NT], F32)
            xprojt = consts.tile([P, NT, RK + 2 * S], F32R)
            dtprojt = consts.tile([RK, DIN], F32R)
            dtb = consts.tile([P, NT], F32)
            acol = consts.tile([P, NT * S], F32)
            dcol = consts.tile([P, NT], F32)
            wopt = consts.tile([P, NT, P], F32R)
            ident = consts.tile([P, P], BF16)
            # critical path first: pair0 input, then xm-chain weights
            xh_tile0 = acts.tile([P, L], F32R, tag="xh0")
            xh_tile1 = acts.tile([P, L], F32R, tag="xh1")
            xh_t = {0: xh_tile0, 1: xh_tile1}
            nc.sync.dma_start(xh_t[0][:], d_xh[:, 0, :])
            for sb, dr in [(wxt, d_wxt), (convd, d_convd), (convb, d_convb),
                           (bias4, d_bias4), (xprojt, d_xprojt),
                           (dtprojt, d_dtprojt), (dtb, d_dtb), (acol, d_acol)]:
                nc.sync.dma_start(sb[:], dr[:])
            nc.sync.dma_start(xh_t[1][:], d_xh[:, 1, :])
            for sb, dr in [(ident, d_ident), (wzt, d_wzt),
                           (dcol, d_dcol), (wopt, d_wopt)]:
                nc.sync.dma_start(sb[:], dr[:])

            zs_t, xc_t, dtr_t, bc_t, dt_t, up_t, yacc_t, ym_t = ({} for _ in range(8))

            # ======== phase A (ACT table: silu_and_others) ========
            def phaseA(pair):
                xh = xh_t[pair]

                # xm path (padded with 3 zero cols for causal conv)
                xmp = acts.tile([P, NT, KC - 1 + L], F32R, tag=f"xmp{pair}")
                for dt_i in range(NT):
                    nc.vector.memset(xmp[:, dt_i, 0:KC - 1].bitcast(F32), 0.0)
                    for ch in range(NCH):
                        ps = psum.tile([P, 512], F32, tag="mm")
                        nc.tensor.matmul(ps[:], _r(wxt[:, dt_i * P:(dt_i + 1) * P]),
                                         _r(xh[:, ch * 512:(ch + 1) * 512]))
                        nc.scalar.activation(
                            xmp[:, dt_i, KC - 1 + ch * 512:KC - 1 + (ch + 1) * 512],
                            ps[:], AF.Identity, bias=bias4[:, dt_i:dt_i + 1])

                # causal depthwise conv (4 shifted diag matmuls) + silu
                xc = acts.tile([P, NT, L], F32R, tag=f"xc{pair}")
                xc_t[pair] = xc
                for dt_i in range(NT):
                    for ch in range(NCH):
                        ps = psum.tile([P, 512], F32, tag="mm")
                        for k in range(KC):
                            nc.tensor.matmul(
                                ps[:], _r(convd[:, dt_i * KC + k, :]),
                                _r(xmp[:, dt_i, ch * 512 + k:ch * 512 + k + 512]),
                                start=(k == 0), stop=(k == KC - 1))
                        nc.scalar.activation(xc[:, dt_i, ch * 512:(ch + 1) * 512],
                                             ps[:], AF.Silu,
                                             bias=convb[:, dt_i:dt_i + 1])

                # x_proj: dbc = xproj @ xc
                dtr = acts.tile([RK, L], F32R, tag=f"dtr{pair}")
                bc = acts.tile([2 * S, L], BF16, tag=f"bc{pair}")
                dtr_t[pair], bc_t[pair] = dtr, bc
                for ch in range(NCH):
                    ps = psum.tile([RK + 2 * S, 512], F32, tag="mmdbc")
                    for ki in range(NT):
                        nc.tensor.matmul(ps[:], _r(xprojt[:, ki, :]),
                                         _r(xc[:, ki, ch * 512:(ch + 1) * 512]),
                                         start=(ki == 0), stop=(ki == NT - 1))
                    # host reordered x_proj rows: [B(16), C(16), dt_r(8)]
                    nc.scalar.copy(bc[:, ch * 512:(ch + 1) * 512], ps[0:2 * S, :])
                    nc.scalar.copy(dtr[:, ch * 512:(ch + 1) * 512],
                                   ps[2 * S:2 * S + RK, :])

                # z path last (independent until phase C): silu(Wz@xh + bz)
                zs = acts.tile([P, NT, L], F32, tag=f"zs{pair}")
                zs_t[pair] = zs
                for dt_i in range(NT):
                    for ch in range(NCH):
                        ps = psum.tile([P, 512], F32, tag="mm")
                        nc.tensor.matmul(ps[:], _r(wzt[:, dt_i * P:(dt_i + 1) * P]),
                                         _r(xh[:, ch * 512:(ch + 1) * 512]))
                        nc.scalar.activation(zs[:, dt_i, ch * 512:(ch + 1) * 512],
                                             ps[:], AF.Silu,
                                             bias=bias4[:, 2 + dt_i:3 + dt_i])

            # ======== phase B (ACT table: natural_log_exp_and_others) ========
            def phaseB(pair):
                # dt_proj ; softplus(x) = ln(exp(x + b) + 1)
                dt_sb = acts.tile([P, NT, L], F32, tag=f"dt{pair}")
                dt_t[pair] = dt_sb
                for dt_i in range(NT):
                    for ch in range(NCH):
                        ps = psum.tile([P, 512], F32, tag="mm")
                        nc.tensor.matmul(ps[:], _r(dtprojt[:, dt_i * P:(dt_i + 1) * P]),
                                         _r(dtr_t[pair][:, ch * 512:(ch + 1) * 512]))
                        nc.scalar.activation(dt_sb[:, dt_i, ch * 512:(ch + 1) * 512],
                                             ps[:], AF.Exp,
                                             bias=dtb[:, dt_i:dt_i + 1])
                    nc.scalar.activation(dt_sb[:, dt_i, :], dt_sb[:, dt_i, :],
                                         AF.Ln, bias=1.0)

                # u' = dt * xc (bf16)
                up = acts.tile([P, NT, L], BF16, tag=f"up{pair}")
                up_t[pair] = up
                for dt_i in range(NT):
                    nc.vector.tensor_mul(up[:, dt_i, :], dt_t[pair][:, dt_i, :],
                                         xc_t[pair][:, dt_i, :].bitcast(F32))

                # bounce B/C rows to DRAM so partition-broadcast DMA reads work
                bcd = dramp.tile([2 * S, L], BF16, tag="bcd")
                nc.sync.dma_start(bcd[:], bc_t[pair][:])

                # selective scan over s; y accumulated on PE (identity matmuls
                # into PSUM) to keep DVE free for bB/scan/g.
                yps = ypsum.tile([P, NT, L], F32, tag="yps")
                yacc_t[pair] = yps
                dt2 = dt_t[pair]
                for s in range(S):
                    br = brep_pool.tile([P, NT, L], BF16, tag="br")
                    cr = crep_pool.tile([P, NT, L], BF16, tag="cr")
                    bsrc = bcd[s:s + 1, :].unsqueeze(1).to_broadcast([P, NT, L])
                    csrc = bcd[S + s:S + s + 1, :].unsqueeze(1).to_broadcast([P, NT, L])
                    nc.sync.dma_start(br[:], bsrc)
                    nc.sync.dma_start(cr[:], csrc)
                    dA = sblk.tile([P, NT, L], BF16, tag="dA")
                    bB = sblk.tile([P, NT, L], BF16, tag="bB")
                    h = hblk.tile([P, NT, L], BF16, tag="h")
                    g = hblk.tile([P, NT, L], BF16, tag="g")
                    if fused_exp:
                        nc.scalar.activation(
                            dA[:].rearrange("p a l -> p (a l)"),
                            dt2[:].rearrange("p a l -> p (a l)"), AF.Exp,
                            scale=acol[:, s:s + 1])
                    else:
                        for dt_i in range(NT):
                            nc.scalar.activation(
                                dA[:, dt_i, :], dt2[:, dt_i, :], AF.Exp,
                                scale=acol[:, dt_i * S + s:dt_i * S + s + 1])
                    nc.vector.tensor_mul(bB[:], up[:], br[:])
                    for dt_i in range(NT):
                        nc.vector.tensor_tensor_scan(
                            h[:, dt_i, :], dA[:, dt_i, :], bB[:, dt_i, :],
                            0.0, OP.mult, OP.add)
                    nc.vector.tensor_mul(g[:], h[:], cr[:])
                    for dt_i in range(NT):
                        for ch in range(NCH):
                            nc.tensor.matmul(
                                yps[:, dt_i, ch * 512:(ch + 1) * 512], ident[:],
                                g[:, dt_i, ch * 512:(ch + 1) * 512],
                                start=(s == 0), stop=(s == S - 1))

            # ======== phase C (output; Copy is in every table) ========
            def phaseC(pair):
                # y = yacc + xc*D ; ym = y * silu(z)
                ym = acts.tile([P, NT, L], F32R, tag=f"ym{pair}")
                for dt_i in range(NT):
                    nc.vector.scalar_tensor_tensor(
                        ym[:, dt_i, :],
                        xc_t[pair][:, dt_i, :].bitcast(F32),
                        dcol[:, dt_i:dt_i + 1],
                        yacc_t[pair][:, dt_i, :], OP.mult, OP.add)  # in1 = PSUM
                    nc.vector.tensor_mul(ym[:, dt_i, :],
                                         ym[:, dt_i, :].bitcast(F32),
                                         zs_t[pair][:, dt_i, :])

                # out = Wop @ ym
                osb = outp.tile([P, L], F32, tag="osb")
                for ch in range(NCH):
                    ps = psum.tile([P, 512], F32, tag="mm")
                    for ki in range(NT):
                        nc.tensor.matmul(ps[:], _r(wopt[:, ki, :]),
                                         _r(ym[:, ki, ch * 512:(ch + 1) * 512]),
                                         start=(ki == 0), stop=(ki == NT - 1))
                    nc.scalar.copy(osb[:, ch * 512:(ch + 1) * 512], ps[:])
                    nc.sync.dma_start(d_out[pair][:, ch * 512:(ch + 1) * 512],
                                      osb[:, ch * 512:(ch + 1) * 512])

            # pair0 full depth first so its scan starts ASAP; pair1 overlaps
            phaseA(0)
            phaseB(0)
            phaseA(1)
            phaseB(1)
            phaseC(0)
            phaseC(1)

    nc.compile()
    return nc


_CACHE = {}


def _get_program(fused_exp=False):
    key = ("nc", fused_exp)
    if key not in _CACHE:
        _CACHE[key] = build_program(fused_exp)
    return _CACHE[key]


def run_traced(**inputs):
    """test-only helper: run once more with NTFF tracing, return exec_time_ns."""
    in_maps = _CACHE.get("in_maps")
    if in_maps is None:
        kernel(**inputs)
        in_maps = _CACHE["in_maps"]
    res = run_bass_kernel_spmd(_get_program(_CACHE.get("fused", False)), in_maps,
                               core_ids=list(range(NCORES)), trace=True)
    return res.exec_time_ns


def kernel(**inputs):
    x = np.asarray(inputs["x"], np.float32)
    gates = np.asarray(inputs["gates"], np.float32)
    ln_g = np.asarray(inputs["ln_g"], np.float32)
    ln_b = np.asarray(inputs["ln_b"], np.float32)
    in_w = np.asarray(inputs["in_w"], np.float32)
    conv_w = np.asarray(inputs["conv_w"], np.float32)
    conv_b = np.asarray(inputs["conv_b"], np.float32)
    xproj_w = np.asarray(inputs["xproj_w"], np.float32)
    dtproj_w = np.asarray(inputs["dtproj_w"], np.float32)
    dtproj_b = np.asarray(inputs["dtproj_b"], np.float32)
    A_log = np.asarray(inputs["A_log"], np.float32)
    Dp = np.asarray(inputs["D"], np.float32)
    out_w = np.asarray(inputs["out_w"], np.float32)
    proj_w = np.asarray(inputs["proj_w"], np.float32)
    proj_b = np.asarray(inputs["proj_b"], np.float32)

    # ---- host: LayerNorm (stats over C), no affine (folded into weights) ----
    xt = x.reshape(B, C, L).astype(np.float64)
    mu = xt.mean(1, keepdims=True)
    var = ((xt - mu) ** 2).mean(1, keepdims=True)
    xhat = ((xt - mu) / np.sqrt(var + 1e-5)).astype(np.float32)      # (B, C, L)

    A = -np.exp(A_log.astype(np.float64)).astype(np.float32)         # (E, DIN, S)

    in_maps = []
    for core in range(NCORES):
        e = core // 2
        b0 = 2 * (core % 2)
        Wfull = in_w[e] * ln_g[e][None, :]                           # (512, C)
        bfull = in_w[e] @ ln_b[e]                                    # (512,)
        Wx, bx = Wfull[:DIN], bfull[:DIN]
        Wz, bz = Wfull[DIN:], bfull[DIN:]
        Wop = proj_w[e] @ out_w[e]                                   # (C, DIN)

        convd = np.zeros((P, NT * KC, P), np.float32)
        rng = np.arange(P)
        for dt_i in range(NT):
            for k in range(KC):
                convd[rng, dt_i * KC + k, rng] = conv_w[e][dt_i * P:(dt_i + 1) * P, k]

        bias4 = np.stack([bx[:P], bx[P:], bz[:P], bz[P:]], 1)        # (P,4)
        m = {
            "xh": np.stack([xhat[b0], xhat[b0 + 1]], 1).copy(),      # (P,2,L)
            "wxt": Wx.T.copy(),                                      # (C, DIN)
            "wzt": Wz.T.copy(),
            "bias4": bias4,
            "convd": convd,
            "convb": conv_b[e].reshape(NT, P).T.copy(),
            "xprojt": np.concatenate([xproj_w[e][RK:], xproj_w[e][:RK]], 0)
                        .T.reshape(NT, P, RK + 2 * S).transpose(1, 0, 2).copy(),
            "dtprojt": dtproj_w[e].T.copy(),                         # (RK, DIN)
            "dtb": dtproj_b[e].reshape(NT, P).T.copy(),
            "acol": A[e].reshape(NT, P, S).transpose(1, 0, 2).reshape(P, NT * S).copy(),
            "dcol": Dp[e].reshape(NT, P).T.copy(),
            "wopt": Wop.T.reshape(NT, P, P).transpose(1, 0, 2).copy(),
            "ident": None,
        }
        im = {k: np.ascontiguousarray(v, np.float32) for k, v in m.items() if v is not None}
        im["ident"] = _CACHE.setdefault("ident_bf16", np.eye(P, dtype=ml_dtypes.bfloat16))
        in_maps.append(im)

    _CACHE["in_maps"] = in_maps
    # fused exp path valid when A rows are d-independent (per expert)
    fused = bool(np.all(np.abs(A - A[:, :1, :]) <= 1e-6 * np.abs(A[:, :1, :])))
    _CACHE["fused"] = fused
    nc = _get_program(fused)
    res = run_bass_kernel_spmd(nc, in_maps, core_ids=list(range(NCORES)))

    # eo[e, b] = (C, L) expert outputs (without proj_b)
    eo = np.zeros((E, B, C, L), np.float32)
    for core in range(NCORES):
        e = core // 2
        b0 = 2 * (core % 2)
        eo[e, b0] = np.asarray(res.results[core]["o0"])
        eo[e, b0 + 1] = np.asarray(res.results[core]["o1"])

    # ---- host: routing, combine, loss (mirrors reference numerics) ----
    x_gap = x.reshape(B, C, L).mean(-1)                              # (B, C)
    outs = []
    loss = np.float32(0)
    for g in range(4):
        logits = x_gap @ gates[g]
        p = np.exp(logits - logits.max(-1, keepdims=True))
        p = p / p.sum(-1, keepdims=True)
        ti = np.argsort(-p, axis=-1, kind="stable")[:, :TOPK]
        tp = np.take_along_axis(p, ti, -1)
        tp = tp / (tp.sum(-1, keepdims=True) + np.float32(1e-10))
        og = np.zeros((B, C, L), np.float32)
        pb = np.zeros((B, C), np.float32)
        for b in range(B):
            for j in range(TOPK):
                og[b] += tp[b, j] * eo[ti[b, j], b]
                pb[b] += tp[b, j] * proj_b[ti[b, j]]
        og = og + pb[:, :, None]
        outs.append(og.reshape(B, C, Hh, Ww))
        usage = p.mean(0)
        loss = loss + np.var(usage, ddof=1) / (usage.mean() ** 2 + np.float32(1e-10))

    _CACHE["last_exec_time_ns"] = res.exec_time_ns
    return (*outs, np.float32(loss))


# revision 34
# speedup vs baseline: 1.0106x; 1.0106x over previous
"""Trainium2 Bass kernel for nn_MoE_42984032698463 (4-gate MoE over 4 Mamba experts).

Sharding: 16 (expert, batch) pairs -> 8 cores, each core owns 1 expert x 2 batches.
Device per pair: in_proj (PE f32r) -> causal depthwise conv (PE diag-shift matmuls)
-> silu -> x_proj -> dt_proj/softplus -> selective scan via native tensor_tensor_scan
(bf16) -> gated output -> fused out_proj@proj (PE). Host: LayerNorm fold, routing
softmax/top-k, weighted combine, aux loss.
"""
import numpy as np
import ml_dtypes

import concourse.bacc as bacc
import concourse.bass as bass
import concourse.tile as tile
from concourse import mybir
from concourse.bass_utils import run_bass_kernel_spmd

F32 = mybir.dt.float32
F32R = mybir.dt.float32r
BF16 = mybir.dt.bfloat16
AF = mybir.ActivationFunctionType
OP = mybir.AluOpType

# problem dims (hardcoded per contract)
E, B, C, Hh, Ww = 4, 4, 128, 32, 32
L = Hh * Ww            # 1024
DIN = 2 * C            # 256
S, RK, KC = 16, 8, 4   # d_state, dt_rank, d_conv
TOPK = 2
NCORES = 8
P = 128                # partitions
NT = DIN // P          # 2 d-tiles
NCH = L // 512         # 2 N-chunks per matmul row


def _r(ap):
    return ap  # f32r needs producer-side rounding; plain fp32 for now


def _patch_act_tables():
    """Make Exp and Ln resolve to the combined natural_log_exp set (one table
    load instead of thrashing exp_and_others <-> natural_log per softplus).
    Keeps dict order/length so act_func_set_id indices stay valid."""
    if _CACHE.get("tables_patched"):
        return
    import concourse.bacc as _bacc
    real = _bacc.get_activation_tables

    def patched(arch):
        tabs = dict(real(arch))
        if "natural_log_exp_and_others" in tabs:
            for name in ("exp_and_others", "natural_log", "exp_and_friends"):
                if name in tabs:
                    tabs[name] = set()
        return tabs

    _bacc.get_activation_tables = patched
    _CACHE["tables_patched"] = True


def build_program(fused_exp=False):
    _patch_act_tables()
    nc = bacc.Bacc("TRN2", target_bir_lowering=False, debug=False)

    # ---- DRAM I/O (per core) ----
    d_xh = nc.dram_tensor("xh", [P, 2, L], F32R, kind="ExternalInput")
    d_wxt = nc.dram_tensor("wxt", [P, DIN], F32R, kind="ExternalInput")
    d_wzt = nc.dram_tensor("wzt", [P, DIN], F32R, kind="ExternalInput")
    d_bias4 = nc.dram_tensor("bias4", [P, 4], F32, kind="ExternalInput")
    d_convd = nc.dram_tensor("convd", [P, NT * KC, P], F32R, kind="ExternalInput")
    d_convb = nc.dram_tensor("convb", [P, NT], F32, kind="ExternalInput")
    d_xprojt = nc.dram_tensor("xprojt", [P, NT, RK + 2 * S], F32R, kind="ExternalInput")
    d_dtprojt = nc.dram_tensor("dtprojt", [RK, DIN], F32R, kind="ExternalInput")
    d_dtb = nc.dram_tensor("dtb", [P, NT], F32, kind="ExternalInput")
    d_acol = nc.dram_tensor("acol", [P, NT * S], F32, kind="ExternalInput")
    d_dcol = nc.dram_tensor("dcol", [P, NT], F32, kind="ExternalInput")
    d_wopt = nc.dram_tensor("wopt", [P, NT, P], F32, kind="ExternalInput")
    d_ident = nc.dram_tensor("ident", [P, P], BF16, kind="ExternalInput")
    d_out = [nc.dram_tensor(f"o{p}", [P, L], F32, kind="ExternalOutput")
             for p in range(2)]

    with tile.TileContext(nc) as tc:
        with (
            tc.tile_pool(name="consts", bufs=1) as consts,
            tc.tile_pool(name="acts", bufs=1) as acts,
            tc.tile_pool(name="psum", bufs=2, space="PSUM") as psum,
            tc.tile_pool(name="ypsum", bufs=1, space="PSUM") as ypsum,
            tc.tile_pool(name="brep", bufs=3) as brep_pool,
            tc.tile_pool(name="crep", bufs=3) as crep_pool,
            tc.tile_pool(name="sblk", bufs=3) as sblk,
            tc.tile_pool(name="hblk", bufs=3) as hblk,
            tc.tile_pool(name="outp", bufs=2) as outp,
            tc.tile_pool(name="dramp", bufs=2, space="DRAM") as dramp,
        ):
            # ---- load constants ----
            wxt = consts.tile([P, DIN], F32R)
            wzt = consts.tile([P, DIN], F32R)
            bias4 = consts.tile([P, 4], F32)
            convd = consts.tile([P, NT * KC, P], F32R)
            convb = consts.tile([P, NT], F32)
            xprojt = consts.tile([P, NT, RK + 2 * S], F32R)
            dtprojt = consts.tile([RK, DIN], F32R)
            dtb = consts.tile([P, NT], F32)
            acol = consts.tile([P, NT * S], F32)
            dcol = consts.tile([P, NT], F32)
            wopt = consts.tile([P, NT, P], F32)
            ident = consts.tile([P, P], BF16)
            # critical path first: pair0 input, then xm-chain weights
            xh_tile0 = acts.tile([P, L], F32R, tag="xh0")
            xh_tile1 = acts.tile([P, L], F32R, tag="xh1")
            xh_t = {0: xh_tile0, 1: xh_tile1}
            nc.sync.dma_start(xh_t[0][:], d_xh[:, 0, :])
            for sb, dr in [(wxt, d_wxt), (convd, d_convd), (convb, d_convb),
                           (bias4, d_bias4), (xprojt, d_xprojt),
                           (dtprojt, d_dtprojt), (dtb, d_dtb), (acol, d_acol)]:
                nc.sync.dma_start(sb[:], dr[:])
            nc.sync.dma_start(xh_t[1][:], d_xh[:, 1, :])
            for sb, dr in [(ident, d_ident), (wzt, d_wzt),
                           (dcol, d_dcol), (wopt, d_wopt)]:
                nc.sync.dma_start(sb[:], dr[:])

            zs_t, xc_t, dtr_t, bc_t, dt_t, up_t, yacc_t, ym_t = ({} for _ in range(8))

            # ======== phase A (ACT table: silu_and_others) ========
            def phaseA(pair):
                xh = xh_t[pair]

                # xm path (padded with 3 zero cols for causal conv)
                xmp = acts.tile([P, NT, KC - 1 + L], F32R, tag=f"xmp{pair}")
                for dt_i in range(NT):
                    nc.vector.memset(xmp[:, dt_i, 0:KC - 1].bitcast(F32), 0.0)
                    for ch in range(NCH):
                        ps = psum.tile([P, 512], F32, tag="mm")
                        nc.tensor.matmul(ps[:], _r(wxt[:, dt_i * P:(dt_i + 1) * P]),
                                         _r(xh[:, ch * 512:(ch + 1) * 512]))
                        nc.scalar.activation(
                            xmp[:, dt_i, KC - 1 + ch * 512:KC - 1 + (ch + 1) * 512],
                            ps[:], AF.Identity, bias=bias4[:, dt_i:dt_i + 1])

                # causal depthwise conv (4 shifted diag matmuls) + silu
                xc = acts.tile([P, NT, L], F32R, tag=f"xc{pair}")
                xc_t[pair] = xc
                for dt_i in range(NT):
                    for ch in range(NCH):
                        ps = psum.tile([P, 512], F32, tag="mm")
                        for k in range(KC):
                            nc.tensor.matmul(
                                ps[:], _r(convd[:, dt_i * KC + k, :]),
                                _r(xmp[:, dt_i, ch * 512 + k:ch * 512 + k + 512]),
                                start=(k == 0), stop=(k == KC - 1))
                        nc.scalar.activation(xc[:, dt_i, ch * 512:(ch + 1) * 512],
                                             ps[:], AF.Silu,
                                             bias=convb[:, dt_i:dt_i + 1])

                # x_proj: dbc = xproj @ xc
                dtr = acts.tile([RK, L], F32R, tag=f"dtr{pair}")
                bc = acts.tile([2 * S, L], BF16, tag=f"bc{pair}")
                dtr_t[pair], bc_t[pair] = dtr, bc
                for ch in range(NCH):
                    ps = psum.tile([RK + 2 * S, 512], F32, tag="mmdbc")
                    for ki in range(NT):
                        nc.tensor.matmul(ps[:], _r(xprojt[:, ki, :]),
                                         _r(xc[:, ki, ch * 512:(ch + 1) * 512]),
                                         start=(ki == 0), stop=(ki == NT - 1))
                    # host reordered x_proj rows: [B(16), C(16), dt_r(8)]
                    nc.scalar.copy(bc[:, ch * 512:(ch + 1) * 512], ps[0:2 * S, :])
                    nc.scalar.copy(dtr[:, ch * 512:(ch + 1) * 512],
                                   ps[2 * S:2 * S + RK, :])

                # z path last (independent until phase C): silu(Wz@xh + bz)
                zs = acts.tile([P, NT, L], F32, tag=f"zs{pair}")
                zs_t[pair] = zs
                for dt_i in range(NT):
                    for ch in range(NCH):
                        ps = psum.tile([P, 512], F32, tag="mm")
                        nc.tensor.matmul(ps[:], _r(wzt[:, dt_i * P:(dt_i + 1) * P]),
                                         _r(xh[:, ch * 512:(ch + 1) * 512]))
                        nc.scalar.activation(zs[:, dt_i, ch * 512:(ch + 1) * 512],
                                             ps[:], AF.Silu,
                                             bias=bias4[:, 2 + dt_i:3 + dt_i])

            # ======== phase B (ACT table: natural_log_exp_and_others) ========
            def phaseB(pair):
                # dt_proj ; softplus(x) = ln(exp(x + b) + 1)
                dt_sb = acts.tile([P, NT, L], F32, tag=f"dt{pair}")
                dt_t[pair] = dt_sb
                for dt_i in range(NT):
                    for ch in range(NCH):
                        ps = psum.tile([P, 512], F32, tag="mm")
                        nc.tensor.matmul(ps[:], _r(dtprojt[:, dt_i * P:(dt_i + 1) * P]),
                                         _r(dtr_t[pair][:, ch * 512:(ch + 1) * 512]))
                        nc.scalar.activation(dt_sb[:, dt_i, ch * 512:(ch + 1) * 512],
                                             ps[:], AF.Exp,
                                             bias=dtb[:, dt_i:dt_i + 1])
                    nc.scalar.activation(dt_sb[:, dt_i, :], dt_sb[:, dt_i, :],
                                         AF.Ln, bias=1.0)

                # u' = dt * xc (bf16)
                up = acts.tile([P, NT, L], BF16, tag=f"up{pair}")
                up_t[pair] = up
                for dt_i in range(NT):
                    nc.vector.tensor_mul(up[:, dt_i, :], dt_t[pair][:, dt_i, :],
                                         xc_t[pair][:, dt_i, :].bitcast(F32))

                # bounce B/C rows to DRAM so partition-broadcast DMA reads work
                bcd = dramp.tile([2 * S, L], BF16, tag="bcd")
                nc.sync.dma_start(bcd[:], bc_t[pair][:])

                # selective scan over s; y accumulated on PE (identity matmuls
                # into PSUM) to keep DVE free for bB/scan/g.
                yps = ypsum.tile([P, NT, L], F32, tag="yps")
                yacc_t[pair] = yps
                dt2 = dt_t[pair]
                for s in range(S):
                    br = brep_pool.tile([P, NT, L], BF16, tag="br")
                    cr = crep_pool.tile([P, NT, L], BF16, tag="cr")
                    bsrc = bcd[s:s + 1, :].unsqueeze(1).to_broadcast([P, NT, L])
                    csrc = bcd[S + s:S + s + 1, :].unsqueeze(1).to_broadcast([P, NT, L])
                    nc.sync.dma_start(br[:], bsrc)
                    nc.sync.dma_start(cr[:], csrc)
                    dA = sblk.tile([P, NT, L], BF16, tag="dA")
                    bB = sblk.tile([P, NT, L], BF16, tag="bB")
                    h = hblk.tile([P, NT, L], BF16, tag="h")
                    g = hblk.tile([P, NT, L], BF16, tag="g")
                    if fused_exp:
                        nc.scalar.activation(
                            dA[:].rearrange("p a l -> p (a l)"),
                            dt2[:].rearrange("p a l -> p (a l)"), AF.Exp,
                            scale=acol[:, s:s + 1])
                    else:
                        for dt_i in range(NT):
                            nc.scalar.activation(
                                dA[:, dt_i, :], dt2[:, dt_i, :], AF.Exp,
                                scale=acol[:, dt_i * S + s:dt_i * S + s + 1])
                    nc.vector.tensor_mul(bB[:], up[:], br[:])
                    for dt_i in range(NT):
                        nc.vector.tensor_tensor_scan(
                            h[:, dt_i, :], dA[:, dt_i, :], bB[:, dt_i, :],
                            0.0, OP.mult, OP.add)
                    nc.vector.tensor_mul(g[:], h[:], cr[:])
                    for dt_i in range(NT):
                        for ch in range(NCH):
                            nc.tensor.matmul(
                                yps[:, dt_i, ch * 512:(ch + 1) * 512], ident[:],
                                g[:, dt_i, ch * 512:(ch + 1) * 512],
                                start=(s == 0), stop=(s == S - 1))

            # ======== phase C (output; Copy is in every table) ========
            def phaseC(pair):
                # y = yacc + xc*D ; ym = y * silu(z)
                ym = acts.tile([P, NT, L], F32, tag=f"ym{pair}")
                for dt_i in range(NT):
                    nc.vector.scalar_tensor_tensor(
                        ym[:, dt_i, :],
                        xc_t[pair][:, dt_i, :].bitcast(F32),
                        dcol[:, dt_i:dt_i + 1],
                        yacc_t[pair][:, dt_i, :], OP.mult, OP.add)  # in1 = PSUM
                    nc.vector.tensor_mul(ym[:, dt_i, :], ym[:, dt_i, :],
                                         zs_t[pair][:, dt_i, :])

                # out = Wop @ ym
                osb = outp.tile([P, L], F32, tag="osb")
                for ch in range(NCH):
                    ps = psum.tile([P, 512], F32, tag="mm")
                    for ki in range(NT):
                        nc.tensor.matmul(ps[:], _r(wopt[:, ki, :]),
                                         _r(ym[:, ki, ch * 512:(ch + 1) * 512]),
                                         start=(ki == 0), stop=(ki == NT - 1))
                    nc.scalar.copy(osb[:, ch * 512:(ch + 1) * 512], ps[:])
                    nc.sync.dma_start(d_out[pair][:, ch * 512:(ch + 1) * 512],
                                      osb[:, ch * 512:(ch + 1) * 512])

            # pair0 full depth first so its scan starts ASAP; pair1 overlaps
            phaseA(0)
            phaseB(0)
            phaseA(1)
            phaseB(1)
            phaseC(0)
            phaseC(1)

    nc.compile()
    return nc


_CACHE = {}


def _get_program(fused_exp=False):
    key = ("nc", fused_exp)
    if key not in _CACHE:
        _CACHE[key] = build_program(fused_exp)
    return _CACHE[key]


def run_traced(**inputs):
    """test-only helper: run once more with NTFF tracing, return exec_time_ns."""
    in_maps = _CACHE.get("in_maps")
    if in_maps is None:
        kernel(**inputs)
        in_maps = _CACHE["in_maps"]
    res = run_bass_kernel_spmd(_get_program(_CACHE.get("fused", False)), in_maps,
                               core_ids=list(range(NCORES)), trace=True)
    return res.exec_time_ns


def kernel(**inputs):
    x = np.asarray(inputs["x"], np.float32)
    gates = np.asarray(inputs["gates"], np.float32)
    ln_g = np.asarray(inputs["ln_g"], np.float32)
    ln_b = np.asarray(inputs["ln_b"], np.float32)
    in_w = np.asarray(inputs["in_w"], np.float32)
    conv_w = np.asarray(inputs["conv_w"], np.float32)
    conv_b = np.asarray(inputs["conv_b"], np.float32)
    xproj_w = np.asarray(inputs["xproj_w"], np.float32)
    dtproj_w = np.asarray(inputs["dtproj_w"], np.float32)
    dtproj_b = np.asarray(inputs["dtproj_b"], np.float32)
    A_log = np.asarray(inputs["A_log"], np.float32)
    Dp = np.asarray(inputs["D"], np.float32)
    out_w = np.asarray(inputs["out_w"], np.float32)
    proj_w = np.asarray(inputs["proj_w"], np.float32)
    proj_b = np.asarray(inputs["proj_b"], np.float32)

    # ---- host: LayerNorm (stats over C), no affine (folded into weights) ----
    xt = x.reshape(B, C, L).astype(np.float64)
    mu = xt.mean(1, keepdims=True)
    var = ((xt - mu) ** 2).mean(1, keepdims=True)
    xhat = ((xt - mu) / np.sqrt(var + 1e-5)).astype(np.float32)      # (B, C, L)

    A = -np.exp(A_log.astype(np.float64)).astype(np.float32)         # (E, DIN, S)

    in_maps = []
    for core in range(NCORES):
        e = core // 2
        b0 = 2 * (core % 2)
        Wfull = in_w[e] * ln_g[e][None, :]                           # (512, C)
        bfull = in_w[e] @ ln_b[e]                                    # (512,)
        Wx, bx = Wfull[:DIN], bfull[:DIN]
        Wz, bz = Wfull[DIN:], bfull[DIN:]
        Wop = proj_w[e] @ out_w[e]                                   # (C, DIN)

        convd = np.zeros((P, NT * KC, P), np.float32)
        rng = np.arange(P)
        for dt_i in range(NT):
            for k in range(KC):
                convd[rng, dt_i * KC + k, rng] = conv_w[e][dt_i * P:(dt_i + 1) * P, k]

        bias4 = np.stack([bx[:P], bx[P:], bz[:P], bz[P:]], 1)        # (P,4)
        m = {
            "xh": np.stack([xhat[b0], xhat[b0 + 1]], 1).copy(),      # (P,2,L)
            "wxt": Wx.T.copy(),                                      # (C, DIN)
            "wzt": Wz.T.copy(),
            "bias4": bias4,
            "convd": convd,
            "convb": conv_b[e].reshape(NT, P).T.copy(),
            "xprojt": np.concatenate([xproj_w[e][RK:], xproj_w[e][:RK]], 0)
                        .T.reshape(NT, P, RK + 2 * S).transpose(1, 0, 2).copy(),
            "dtprojt": dtproj_w[e].T.copy(),                         # (RK, DIN)
            "dtb": dtproj_b[e].reshape(NT, P).T.copy(),
            "acol": A[e].reshape(NT, P, S).transpose(1, 0, 2).reshape(P, NT * S).copy(),
            "dcol": Dp[e].reshape(NT, P).T.copy(),
            "wopt": Wop.T.reshape(NT, P, P).transpose(1, 0, 2).copy(),
            "ident": None,
        }
        im = {k: np.ascontiguousarray(v, np.float32) for k, v in m.items() if v is not None}
        im["ident"] = _CACHE.setdefault("ident_bf16", np.eye(P, dtype=ml_dtypes.bfloat16))
        in_maps.append(im)

    _CACHE["in_maps"] = in_maps
    # fused exp path valid when A rows are d-independent (per expert)
    fused = bool(np.all(np.abs(A - A[:, :1, :]) <= 1e-6 * np.abs(A[:, :1, :])))
    _CACHE["fused"] = fused
    nc = _get_program(fused)
    res = run_bass_kernel_spmd(nc, in_maps, core_ids=list(range(NCORES)))

    # eo[e, b] = (C, L) expert outputs (without proj_b)
    eo = np.zeros((E, B, C, L), np.float32)
    for core in range(NCORES):
        e = core // 2
        b0 = 2 * (core % 2)
        eo[e, b0] = np.asarray(res.results[core]["o0"])
        eo[e, b0 + 1] = np.asarray(res.results[core]["o1"])

    # ---- host: routing, combine, loss (mirrors reference numerics) ----
    x_gap = x.reshape(B, C, L).mean(-1)                              # (B, C)
    outs = []
    loss = np.float32(0)
    for g in range(4):
        logits = x_gap @ gates[g]
        p = np.exp(logits - logits.max(-1, keepdims=True))
        p = p / p.sum(-1, keepdims=True)
        ti = np.argsort(-p, axis=-1, kind="stable")[:, :TOPK]
        tp = np.take_along_axis(p, ti, -1)
        tp = tp / (tp.sum(-1, keepdims=True) + np.float32(1e-10))
        og = np.zeros((B, C, L), np.float32)
        pb = np.zeros((B, C), np.float32)
        for b in range(B):
            for j in range(TOPK):
                og[b] += tp[b, j] * eo[ti[b, j], b]
                pb[b] += tp[b, j] * proj_b[ti[b, j]]
        og = og + pb[:, :, None]
        outs.append(og.reshape(B, C, Hh, Ww))
        usage = p.mean(0)
        loss = loss + np.var(usage, ddof=1) / (usage.mean() ** 2 + np.float32(1e-10))

    _CACHE["last_exec_time_ns"] = res.exec_time_ns
    return (*outs, np.float32(loss))
